# revision 7
# baseline (speedup 1.0000x reference)
"""Chunk-causal Whisper attention (B=4, T=1500, D=1024, H=16) on 8 NeuronCores.

Sharding: core c = (batch b = c//2, head-half hh = c%2). Each core runs one
batch element with 8 of the 16 heads (512 of 1024 channels). All matmul
operands are bf16 (PSUM accumulation stays fp32); rel tolerance 2e-2 leaves
ample room. All on-chip tensors are kept transposed: scoresT[k,q] = K @ Q^T
per head, so the softmax reduction runs along the partition (key) axis.

Phase 2 processes heads in pairs (even head on partitions 0-63, odd head on
64-127 of each 128-channel chunk, matching the Q/K projection layout):
  - scores: two row-tiled matmuls (64-row contraction each) run concurrently
    on the two halves of the PE array into one 2-bank PSUM tile.
  - exp: one ACT instruction over the pair, trimmed to the query range that
    has any allowed key in the block (host-precomputed qlo per block).
  - mask: multiplicative {1,0} bf16 mask applied AFTER exp (fast 2-byte DVE
    mode); it also zeroes the stale sub-qlo range of the ex tile.
  - PV + denominator: per head, two col-tiled matmuls run concurrently:
    V (cols 0-63) -> pv tile, ones (cols 64-127) -> den tile broadcast over
    64 partitions. Even head lands at partitions 0-63, odd at 64-127, so one
    reciprocal + one multiply normalizes the whole pair at matching lanes.
Scores are emitted one kc-block ahead of PV so the PE never starves (keeps
the HAM clock gate warm at 2.4 GHz; the fp32r baseline sat throttled at
1.2 GHz for the entire attention phase).

T is padded 1500 -> 1536 with zeros (hsT columns): padded keys contribute
nothing (V rows are zero and partial-block masks zero their exp values);
padded query columns are finite garbage and dropped on the host. bv folds
through softmax (probabilities sum to 1), so the host adds (bv @ Wo + bo).
"""

import sys

import numpy as np
import ml_dtypes

if "/opt/trn_rl_repo" not in sys.path:
    sys.path.insert(0, "/opt/trn_rl_repo")

import concourse.tile as tile  # noqa: E402
from concourse import bacc, mybir  # noqa: E402
import concourse.bass_utils as bass_utils  # noqa: E402

B, T, D = 4, 1500, 1024
H, HD = 16, 64
CHUNK, LOOK = 100, 50
TP = 1536          # padded sequence length
CH = 512           # channels per core (8 heads)
HL = 8             # heads per core
NPAIR = HL // 2
NCORES = 8
SCALE = HD ** -0.5
QB = 512           # query block (matmul moving free dim)
KB = 128           # key block (contraction tile)
NQC = TP // QB     # 3
NKC = TP // KB     # 12
NDC = D // 128     # 8
NCC = CH // 128    # 4
NOC = D // 128     # 8 output-column chunks
F32 = mybir.dt.float32
BF16 = mybir.dt.bfloat16
BF16NP = ml_dtypes.bfloat16


def _jmax(i):
    """Largest key index query i may attend to."""
    return max((i // CHUNK) * CHUNK + CHUNK - 1, i + LOOK)


def _classify():
    """Per (qc, kc) block: status 'skip' | 'full' | mask-index, plus the
    first query column with any allowed key (qlo, floored to 8)."""
    status = {}
    qlos = {}
    masks = []
    for qc in range(NQC):
        q0 = qc * QB
        hi = max(_jmax(min(q0 + ii, T - 1)) for ii in range(QB))
        lo = _jmax(q0)
        for kc in range(NKC):
            k0 = kc * KB
            if k0 > hi:
                status[(qc, kc)] = "skip"
                continue
            if k0 + KB - 1 <= lo and k0 + KB <= T:
                status[(qc, kc)] = "full"
                qlos[(qc, kc)] = 0
                continue
            m = np.zeros((KB, QB), np.float32)
            qlo = QB
            for ii in range(QB):
                i = min(q0 + ii, T - 1)  # padded queries reuse the last row
                n_ok = min(min(_jmax(i), T - 1) + 1 - k0, KB)
                if n_ok > 0:
                    m[:n_ok, ii] = 1.0
                    qlo = min(qlo, ii)
            status[(qc, kc)] = len(masks)
            qlos[(qc, kc)] = (qlo // 8) * 8
            masks.append(m)
    return status, qlos, masks


_STATUS, _QLO, _MASKS = _classify()
NPART = len(_MASKS)

# [128, NPART, 2, QB] multiplicative bf16 mask, duplicated for the head pair.
_EXPMASK = np.ascontiguousarray(
    np.broadcast_to(np.stack(_MASKS)[:, :, None, :], (NPART, KB, 2, QB))
    .transpose(1, 0, 2, 3)
).astype(BF16NP)


def _build():
    nc = bacc.Bacc("TRN2", target_bir_lowering=False, debug=False)
    hsT = nc.dram_tensor("hsT", [D, TP], BF16, kind="ExternalInput")[:]
    wq = nc.dram_tensor("wq", [D, CH], BF16, kind="ExternalInput")[:]
    wk = nc.dram_tensor("wk", [D, CH], BF16, kind="ExternalInput")[:]
    wv = nc.dram_tensor("wv", [D, CH], BF16, kind="ExternalInput")[:]
    bqs = nc.dram_tensor("bqs", [CH], F32, kind="ExternalInput")[:]
    wo = nc.dram_tensor("wo", [CH, D], BF16, kind="ExternalInput")[:]
    maskT = nc.dram_tensor("maskT", [KB, NPART, 2, QB], BF16,
                           kind="ExternalInput")[:]
    outT = nc.dram_tensor("outT", [D, TP], F32, kind="ExternalOutput")[:]

    hsT_r = hsT.rearrange("(a p) t -> a p t", p=128)
    wq_r = wq.rearrange("(a p) c -> a p c", p=128)
    wk_r = wk.rearrange("(a p) c -> a p c", p=128)
    wv_r = wv.rearrange("(a p) c -> a p c", p=128)
    wo_r = wo.rearrange("(a p) o -> a p o", p=128)
    outT_r = outT.rearrange("(a p) t -> a p t", p=128)

    ExpF = mybir.ActivationFunctionType.Exp

    with tile.TileContext(nc) as tc, tc.tile_pool(name="per", bufs=1) as per:
        KT = per.tile([128, NCC, TP], BF16)        # K^T: [c, cc, t]
        QT = per.tile([128, NCC, TP], BF16)        # Q^T (scale+bias folded)
        Vx = per.tile([128, NKC, HL, HD], BF16)    # V: [k, kc, h, hd]
        AT = per.tile([128, NCC, TP], BF16)        # normalized attnT
        ones64 = per.tile([128, HD], BF16)
        bq_sb = per.tile([128, NCC], F32)
        mk_sb = per.tile([128, NPART, 2, QB], BF16)
        wo_sb = per.tile([128, NCC, D], BF16)
        nc.vector.memset(ones64[:], 1.0)
        nc.sync.dma_start(bq_sb[:], bqs.rearrange("(a p) -> p a", p=128))
        # Mask + Wo land early so the phase-1 -> 2 transition never waits
        # on DMA (the 3.6MB mask transfer stalled the PE ~10us when issued
        # at the start of phase 2).
        nc.sync.dma_start(mk_sb[:], maskT[:])
        for cc in range(NCC):
            nc.sync.dma_start(wo_sb[:, cc, :], wo_r[cc])

        # ---------------- phase 1: Q/K/V projections ----------------
        with tc.tile_pool(name="p1", bufs=1) as p1, \
             tc.tile_pool(name="w", bufs=2) as wp, \
             tc.tile_pool(name="ps1", bufs=2, space="PSUM") as ps1:
            hs_sb = p1.tile([128, NDC, TP], BF16)
            wk_sb = wp.tile([128, NDC, CH], BF16, tag="w")
            for dc in range(NDC):
                nc.sync.dma_start(hs_sb[:, dc, :], hsT_r[dc])
                nc.sync.dma_start(wk_sb[:, dc, :], wk_r[dc])

            # K^T[c, t] = sum_d Wk[d, c] * hsT[d, t]
            for tb in range(NQC):
                ts = slice(tb * QB, (tb + 1) * QB)
                pss = [ps1.tile([128, QB], F32, tag=f"pp{cc}", name=f"pp{cc}")
                       for cc in range(NCC)]
                for dc in range(NDC):
                    for cc in range(NCC):
                        nc.tensor.matmul(
                            pss[cc][:],
                            wk_sb[:, dc, cc * 128:(cc + 1) * 128],
                            hs_sb[:, dc, ts],
                            start=(dc == 0), stop=(dc == NDC - 1),
                        )
                for cc in range(NCC):
                    nc.scalar.copy(KT[:, cc, ts], pss[cc][:])

            # Q^T[c, t] (wq pre-scaled by 1/sqrt(hd) on host; bias added here)
            wq_sb = wp.tile([128, NDC, CH], BF16, tag="w")
            for dc in range(NDC):
                nc.sync.dma_start(wq_sb[:, dc, :], wq_r[dc])
            for tb in range(NQC):
                ts = slice(tb * QB, (tb + 1) * QB)
                pss = [ps1.tile([128, QB], F32, tag=f"pp{cc}", name=f"pp{cc}")
                       for cc in range(NCC)]
                for dc in range(NDC):
                    for cc in range(NCC):
                        nc.tensor.matmul(
                            pss[cc][:],
                            wq_sb[:, dc, cc * 128:(cc + 1) * 128],
                            hs_sb[:, dc, ts],
                            start=(dc == 0), stop=(dc == NDC - 1),
                        )
                for cc in range(NCC):
                    nc.vector.tensor_scalar_add(
                        QT[:, cc, ts], pss[cc][:], bq_sb[:, cc:cc + 1]
                    )

            # V[t, c] = sum_d hs[t, d] * Wv[d, c]  (bias bv folded to host)
            wv_sb = wp.tile([128, NDC, CH], BF16, tag="w")
            for dc in range(NDC):
                nc.sync.dma_start(wv_sb[:, dc, :], wv_r[dc])
            for tk in range(NKC):
                psv = ps1.tile([128, CH], F32, tag="pp0", name="ppv")
                for dc in range(NDC):
                    nc.tensor.matmul(
                        psv[:],
                        hs_sb[:, dc, tk * KB:(tk + 1) * KB],
                        wv_sb[:, dc, :],
                        start=(dc == 0), stop=(dc == NDC - 1),
                    )
                eng = nc.scalar if tk % 2 == 0 else nc.vector
                if tk % 2 == 0:
                    nc.scalar.copy(
                        Vx[:, tk, :, :],
                        psv[:].rearrange("p (h d) -> p h d", d=HD),
                    )
                else:
                    nc.vector.tensor_copy(
                        Vx[:, tk, :, :],
                        psv[:].rearrange("p (h d) -> p h d", d=HD),
                    )

        # --------- phases 2+3: attention, out-proj interleaved ---------
        # qc-outer so AT[:, :, qs] completes per query block; each query
        # block's 32 output-projection matmuls are then drip-fed into the
        # NEXT query block's scalar-bound step stream (the PE has slack
        # there), instead of running as a serial PE-only tail.
        with tc.tile_pool(name="ex", bufs=3) as ep, \
             tc.tile_pool(name="rcp", bufs=2) as rp, \
             tc.tile_pool(name="fin", bufs=2) as fp, \
             tc.tile_pool(name="ps_s", bufs=2, space="PSUM") as ps_s, \
             tc.tile_pool(name="ps_pv", bufs=2, space="PSUM") as ps_pv, \
             tc.tile_pool(name="ps_dn", bufs=2, space="PSUM") as ps_dn:

            def make_p3(qc):
                """Return per-oc emitters for query block qc's out-proj."""
                qs = slice(qc * QB, (qc + 1) * QB)
                fin = fp.tile([128, NOC, QB], F32, tag="fin", name="fin")

                def emit_oc(oc):
                    pot = ps_s.tile([128, 2, QB], F32, tag="ss", name="po")
                    po = pot[:, 0, :]
                    for cc in range(NCC):
                        nc.tensor.matmul(
                            po,
                            wo_sb[:, cc, oc * 128:(oc + 1) * 128],
                            AT[:, cc, qs],
                            start=(cc == 0), stop=(cc == NCC - 1),
                        )
                    nc.vector.tensor_copy(fin[:, oc, :], po)
                    nc.sync.dma_start(outT_r[oc][:, qs], fin[:, oc, :])

                return [lambda oc=oc: emit_oc(oc) for oc in range(NOC)]

            deferred = []
            for qc in range(NQC):
                qs = slice(qc * QB, (qc + 1) * QB)
                kcs = [kc for kc in range(NKC)
                       if _STATUS[(qc, kc)] != "skip"]
                n = len(kcs)
                # spread the previous block's out-proj over this block's steps
                nstep = NPAIR * n
                p3_at = {}
                if deferred:
                    stride = max(2, nstep // len(deferred))
                    for j, em in enumerate(deferred):
                        key = min(j * stride + 1, nstep - 1)
                        p3_at.setdefault(key, []).append(em)
                step_no = 0

                for pair in range(NPAIR):
                    cc = pair
                    pv = ps_pv.tile([128, QB], F32)
                    dn = ps_dn.tile([128, QB], F32)
                    ss_t = [None] * n
                    ex_t = [None] * n

                    def emit_s(idx):
                        kc = kcs[idx]
                        qlo = _QLO[(qc, kc)]
                        ss = ps_s.tile([128, 2, QB], F32, tag="ss", name="ss")
                        ss_t[idx] = ss
                        qsl = slice(qc * QB + qlo, (qc + 1) * QB)
                        ks = slice(kc * KB, (kc + 1) * KB)
                        nc.tensor.matmul(
                            ss[:, 0, qlo:], KT[0:64, cc, ks],
                            QT[0:64, cc, qsl], start=True, stop=True,
                        )
                        nc.tensor.matmul(
                            ss[:, 1, qlo:], KT[64:128, cc, ks],
                            QT[64:128, cc, qsl], start=True, stop=True,
                        )

                    def emit_e(idx):
                        kc = kcs[idx]
                        st = _STATUS[(qc, kc)]
                        qlo = _QLO[(qc, kc)]
                        ex = ep.tile([128, 2, QB], BF16, tag="e", name="e")
                        ex_t[idx] = ex
                        nc.scalar.activation(
                            ex[:, :, qlo:], ss_t[idx][:, :, qlo:], ExpF
                        )
                        if st != "full":
                            nc.vector.tensor_mul(
                                ex[:], ex[:], mk_sb[:, st, :, :]
                            )

                    def emit_p(idx):
                        kc = kcs[idx]
                        ex = ex_t[idx]
                        first, last = idx == 0, idx == n - 1
                        for h2 in range(2):
                            h = 2 * pair + h2
                            nc.tensor.matmul(
                                pv[h2 * 64:(h2 + 1) * 64, :],
                                Vx[:, kc, h, :], ex[:, h2, :],
                                start=first, stop=last,
                                skip_group_check=True,
                            )
                            nc.tensor.matmul(
                                dn[h2 * 64:(h2 + 1) * 64, :],
                                ones64[:], ex[:, h2, :],
                                start=first, stop=last,
                                skip_group_check=True,
                            )

                    emit_s(0)
                    emit_e(0)
                    for idx in range(n):
                        if idx + 1 < n:
                            emit_s(idx + 1)
                            emit_e(idx + 1)
                        emit_p(idx)
                        for em in p3_at.pop(step_no, ()):
                            em()
                        step_no += 1

                    rc = rp.tile([128, QB], F32, tag="rc", name="rc")
                    nc.vector.reciprocal_approx_fast(rc[:], dn[:])
                    nc.vector.tensor_mul(AT[:, cc, qs], pv[:], rc[:])

                for ems in p3_at.values():  # any stragglers
                    for em in ems:
                        em()
                deferred = make_p3(qc)

            for em in deferred:  # last query block's out-proj tail
                em()

    nc.finalize()
    return nc


_NC = None


def _get_nc():
    global _NC
    if _NC is None:
        _NC = _build()
    return _NC


def _make_in_maps(hidden_states, Wq, bq, Wk, Wv, Wo):
    hs = np.asarray(hidden_states, np.float32)
    Wq = np.asarray(Wq, np.float32)
    Wk = np.asarray(Wk, np.float32)
    Wv = np.asarray(Wv, np.float32)
    Wo = np.asarray(Wo, np.float32)
    bq = np.asarray(bq, np.float32)

    wq_s = (Wq * np.float32(SCALE)).astype(BF16NP)
    wk_b = Wk.astype(BF16NP)
    wv_b = Wv.astype(BF16NP)
    wo_b = Wo.astype(BF16NP)

    in_maps = []
    for core in range(NCORES):
        b, hh = core // 2, core % 2
        sl = slice(hh * CH, (hh + 1) * CH)
        hsT_pad = np.zeros((D, TP), BF16NP)
        hsT_pad[:, :T] = hs[b].T.astype(BF16NP)
        in_maps.append({
            "hsT": hsT_pad,
            "wq": np.ascontiguousarray(wq_s[:, sl]),
            "wk": np.ascontiguousarray(wk_b[:, sl]),
            "wv": np.ascontiguousarray(wv_b[:, sl]),
            "bqs": np.ascontiguousarray(bq[sl] * np.float32(SCALE)),
            "wo": np.ascontiguousarray(wo_b[sl, :]),
            "maskT": _EXPMASK,
        })
    return in_maps


def _assemble(results, bv, Wo, bo):
    c0 = (np.asarray(bv, np.float32) @ np.asarray(Wo, np.float32)
          + np.asarray(bo, np.float32))
    out = np.empty((B, T, D), np.float32)
    for b in range(B):
        out[b] = (results[2 * b]["outT"][:, :T].T
                  + results[2 * b + 1]["outT"][:, :T].T + c0)
    return out


def kernel(hidden_states, Wq, bq, Wk, Wv, bv, Wo, bo):
    in_maps = _make_in_maps(hidden_states, Wq, bq, Wk, Wv, Wo)
    res = bass_utils.run_bass_kernel_spmd(
        _get_nc(), in_maps, core_ids=list(range(NCORES))
    )
    return _assemble(res.results, bv, Wo, bo)


# revision 12
# speedup vs baseline: 1.0558x; 1.0558x over previous
"""Chunk-causal Whisper attention (B=4, T=1500, D=1024, H=16) on 8 NeuronCores.

Sharding: core c = (batch b = c//2, head-half hh = c%2). Each core runs one
batch element with 8 of the 16 heads (512 of 1024 channels). All matmul
operands are bf16 (PSUM accumulation stays fp32); rel tolerance 2e-2 leaves
ample room. All on-chip tensors are kept transposed: scoresT[k,q] = K @ Q^T
per head, so the softmax reduction runs along the partition (key) axis.

Phase 2 processes heads in pairs (even head on partitions 0-63, odd head on
64-127 of each 128-channel chunk, matching the Q/K projection layout):
  - scores: two row-tiled matmuls (64-row contraction each) run concurrently
    on the two halves of the PE array into one 2-bank PSUM tile.
  - exp: one ACT instruction over the pair, trimmed to the query range that
    has any allowed key in the block (host-precomputed qlo per block).
  - mask: multiplicative {1,0} bf16 mask applied AFTER exp (fast 2-byte DVE
    mode); it also zeroes the stale sub-qlo range of the ex tile.
  - PV + denominator: per head, two col-tiled matmuls run concurrently:
    V (cols 0-63) -> pv tile, ones (cols 64-127) -> den tile broadcast over
    64 partitions. Even head lands at partitions 0-63, odd at 64-127, so one
    reciprocal + one multiply normalizes the whole pair at matching lanes.
Scores are emitted one kc-block ahead of PV so the PE never starves (keeps
the HAM clock gate warm at 2.4 GHz; the fp32r baseline sat throttled at
1.2 GHz for the entire attention phase).

T is padded 1500 -> 1536 with zeros (hsT columns): padded keys contribute
nothing (V rows are zero and partial-block masks zero their exp values);
padded query columns are finite garbage and dropped on the host. bv folds
through softmax (probabilities sum to 1), so the host adds (bv @ Wo + bo).
"""

import sys

import numpy as np
import ml_dtypes

if "/opt/trn_rl_repo" not in sys.path:
    sys.path.insert(0, "/opt/trn_rl_repo")

import concourse.tile as tile  # noqa: E402
from concourse import bacc, mybir  # noqa: E402
import concourse.bass_utils as bass_utils  # noqa: E402

B, T, D = 4, 1500, 1024
H, HD = 16, 64
CHUNK, LOOK = 100, 50
TP = 1536          # padded sequence length
CH = 512           # channels per core (8 heads)
HL = 8             # heads per core
NPAIR = HL // 2
NCORES = 8
SCALE = HD ** -0.5
QB = 512           # query block (matmul moving free dim)
KB = 128           # key block (contraction tile)
NQC = TP // QB     # 3
NKC = TP // KB     # 12
NDC = D // 128     # 8
NCC = CH // 128    # 4
NOC = D // 128     # 8 output-column chunks
F32 = mybir.dt.float32
BF16 = mybir.dt.bfloat16
BF16NP = ml_dtypes.bfloat16


def _jmax(i):
    """Largest key index query i may attend to."""
    return max((i // CHUNK) * CHUNK + CHUNK - 1, i + LOOK)


def _classify():
    """Per (qc, kc) block: status 'skip' | 'full' | mask-index, plus the
    first query column with any allowed key (qlo, floored to 8)."""
    status = {}
    qlos = {}
    masks = []
    for qc in range(NQC):
        q0 = qc * QB
        hi = max(_jmax(min(q0 + ii, T - 1)) for ii in range(QB))
        lo = _jmax(q0)
        for kc in range(NKC):
            k0 = kc * KB
            if k0 > hi:
                status[(qc, kc)] = "skip"
                continue
            if k0 + KB - 1 <= lo and k0 + KB <= T:
                status[(qc, kc)] = "full"
                qlos[(qc, kc)] = 0
                continue
            m = np.zeros((KB, QB), np.float32)
            qlo = QB
            for ii in range(QB):
                i = min(q0 + ii, T - 1)  # padded queries reuse the last row
                n_ok = min(min(_jmax(i), T - 1) + 1 - k0, KB)
                if n_ok > 0:
                    m[:n_ok, ii] = 1.0
                    qlo = min(qlo, ii)
            status[(qc, kc)] = len(masks)
            qlos[(qc, kc)] = (qlo // 8) * 8
            masks.append(m)
    return status, qlos, masks


_STATUS, _QLO, _MASKS = _classify()
NPART = len(_MASKS)

# [128, NPART, 2, QB] multiplicative bf16 mask, duplicated for the head pair.
_EXPMASK = np.ascontiguousarray(
    np.broadcast_to(np.stack(_MASKS)[:, :, None, :], (NPART, KB, 2, QB))
    .transpose(1, 0, 2, 3)
).astype(BF16NP)


def _build():
    nc = bacc.Bacc("TRN2", target_bir_lowering=False, debug=False)
    hsT = nc.dram_tensor("hsT", [D, TP], BF16, kind="ExternalInput")[:]
    wq = nc.dram_tensor("wq", [D, CH], BF16, kind="ExternalInput")[:]
    wk = nc.dram_tensor("wk", [D, CH], BF16, kind="ExternalInput")[:]
    wv = nc.dram_tensor("wv", [D, CH], BF16, kind="ExternalInput")[:]
    bqs = nc.dram_tensor("bqs", [CH], F32, kind="ExternalInput")[:]
    wo = nc.dram_tensor("wo", [CH, D], BF16, kind="ExternalInput")[:]
    maskT = nc.dram_tensor("maskT", [KB, NPART, 2, QB], BF16,
                           kind="ExternalInput")[:]
    outT = nc.dram_tensor("outT", [D, TP], BF16, kind="ExternalOutput")[:]

    hsT_r = hsT.rearrange("(a p) t -> a p t", p=128)
    wq_r = wq.rearrange("(a p) c -> a p c", p=128)
    wk_r = wk.rearrange("(a p) c -> a p c", p=128)
    wv_r = wv.rearrange("(a p) c -> a p c", p=128)
    wo_r = wo.rearrange("(a p) o -> a p o", p=128)
    outT_r = outT.rearrange("(a p) t -> a p t", p=128)

    ExpF = mybir.ActivationFunctionType.Exp

    with tile.TileContext(nc) as tc, tc.tile_pool(name="per", bufs=1) as per:
        KT = per.tile([128, NCC, TP], BF16)        # K^T: [c, cc, t]
        QT = per.tile([128, NCC, TP], BF16)        # Q^T (scale+bias folded)
        Vx = per.tile([128, NKC, HL, HD], BF16)    # V: [k, kc, h, hd]
        AT = per.tile([128, NCC, TP], BF16)        # normalized attnT
        ones64 = per.tile([128, HD], BF16)
        bq_sb = per.tile([128, NCC], F32)
        mk_sb = per.tile([128, NPART, 2, QB], BF16)
        wo_sb = per.tile([128, NCC, D], BF16)
        nc.vector.memset(ones64[:], 1.0)
        nc.sync.dma_start(bq_sb[:], bqs.rearrange("(a p) -> p a", p=128))

        # ---------------- phase 1: K/Q projections ----------------
        # V's projection is deferred into phase 2's instruction stream (the
        # PE has slack there while the scalar engine runs exps), so qc0's
        # exps start ~20us earlier. DMA order matters: hs/wk gate the first
        # matmul; the 3.6MB mask only has to land before qc0's first exp.
        with tc.tile_pool(name="p1", bufs=1) as p1, \
             tc.tile_pool(name="w", bufs=3) as wp:
            hs_sb = p1.tile([128, NDC, TP], BF16)
            wk_sb = wp.tile([128, NDC, CH], BF16, tag="w")
            wq_sb = wp.tile([128, NDC, CH], BF16, tag="w")
            wv_sb = wp.tile([128, NDC, CH], BF16, tag="w")
            for dc in range(NDC):
                nc.sync.dma_start(hs_sb[:, dc, :], hsT_r[dc])
                nc.sync.dma_start(wk_sb[:, dc, :], wk_r[dc])
            for dc in range(NDC):
                nc.sync.dma_start(wq_sb[:, dc, :], wq_r[dc])
            for dc in range(NDC):
                nc.sync.dma_start(wv_sb[:, dc, :], wv_r[dc])
            nc.sync.dma_start(mk_sb[:], maskT[:])
            for cc in range(NCC):
                nc.sync.dma_start(wo_sb[:, cc, :], wo_r[cc])

            with tc.tile_pool(name="ps1", bufs=2, space="PSUM") as ps1:
                # K^T[c, t] = sum_d Wk[d, c] * hsT[d, t]
                for tb in range(NQC):
                    ts = slice(tb * QB, (tb + 1) * QB)
                    pss = [ps1.tile([128, QB], F32, tag=f"pp{cc}",
                                    name=f"pp{cc}") for cc in range(NCC)]
                    for dc in range(NDC):
                        for cc in range(NCC):
                            nc.tensor.matmul(
                                pss[cc][:],
                                wk_sb[:, dc, cc * 128:(cc + 1) * 128],
                                hs_sb[:, dc, ts],
                                start=(dc == 0), stop=(dc == NDC - 1),
                            )
                    for cc in range(NCC):
                        nc.scalar.copy(KT[:, cc, ts], pss[cc][:])

                # Q^T[c, t] (wq pre-scaled by 1/sqrt(hd) on host; bias here)
                for tb in range(NQC):
                    ts = slice(tb * QB, (tb + 1) * QB)
                    pss = [ps1.tile([128, QB], F32, tag=f"pp{cc}",
                                    name=f"pp{cc}") for cc in range(NCC)]
                    for dc in range(NDC):
                        for cc in range(NCC):
                            nc.tensor.matmul(
                                pss[cc][:],
                                wq_sb[:, dc, cc * 128:(cc + 1) * 128],
                                hs_sb[:, dc, ts],
                                start=(dc == 0), stop=(dc == NDC - 1),
                            )
                    for cc in range(NCC):
                        nc.vector.tensor_scalar_add(
                            QT[:, cc, ts], pss[cc][:], bq_sb[:, cc:cc + 1]
                        )

            # --------- phases 2+3: attention, out-proj, V-proj ---------
            # qc-outer so AT[:, :, qs] completes per query block; each query
            # block's 32 output-projection matmuls are then drip-fed into the
            # NEXT query block's scalar-bound step stream (the PE has slack
            # there), instead of running as a serial PE-only tail. The V
            # projection is likewise dripped into qc0's steps (V psum shares
            # the score pool; copies go to vector/gpsimd, keeping scalar for
            # exps).
            with tc.tile_pool(name="ex", bufs=3) as ep, \
                 tc.tile_pool(name="rcp", bufs=2) as rp, \
                 tc.tile_pool(name="fin", bufs=2) as fp, \
                 tc.tile_pool(name="ps_s", bufs=2, space="PSUM") as ps_s, \
                 tc.tile_pool(name="ps_pv", bufs=2, space="PSUM") as ps_pv, \
                 tc.tile_pool(name="ps_dn", bufs=2, space="PSUM") as ps_dn:

                def emit_v(tk):
                    pot = ps_s.tile([128, 2, QB], F32, tag="ss", name="pv1")
                    psv = pot[:, 0, :]
                    for dc in range(NDC):
                        nc.tensor.matmul(
                            psv,
                            hs_sb[:, dc, tk * KB:(tk + 1) * KB],
                            wv_sb[:, dc, :],
                            start=(dc == 0), stop=(dc == NDC - 1),
                            skip_group_check=True,
                        )
                    nc.vector.tensor_copy(
                        Vx[:, tk, :, :],
                        psv.rearrange("p (h d) -> p h d", d=HD),
                    )

                def make_p3(qc):
                    """Return per-oc emitters for query block qc's out-proj."""
                    qs = slice(qc * QB, (qc + 1) * QB)
                    fin = fp.tile([128, NOC, QB], BF16, tag="fin", name="fin")

                    def emit_oc(oc):
                        pot = ps_s.tile([128, 2, QB], F32, tag="ss", name="po")
                        po = pot[:, 0, :]
                        for cc in range(NCC):
                            nc.tensor.matmul(
                                po,
                                wo_sb[:, cc, oc * 128:(oc + 1) * 128],
                                AT[:, cc, qs],
                                start=(cc == 0), stop=(cc == NCC - 1),
                            )
                        nc.vector.tensor_copy(fin[:, oc, :], po)
                        nc.sync.dma_start(outT_r[oc][:, qs], fin[:, oc, :])

                    return [lambda oc=oc: emit_oc(oc) for oc in range(NOC)]

                for tk in range(5):  # qc0's PV blocks need kc <= 4
                    emit_v(tk)
                v_at = {2 * j + 2: tk for j, tk in enumerate(range(5, NKC))}

                deferred = []
                for qc in range(NQC):
                    qs = slice(qc * QB, (qc + 1) * QB)
                    kcs = [kc for kc in range(NKC)
                           if _STATUS[(qc, kc)] != "skip"]
                    n = len(kcs)
                    # spread the previous block's out-proj over these steps
                    nstep = NPAIR * n
                    p3_at = {}
                    if deferred:
                        stride = max(2, nstep // len(deferred))
                        for j, em in enumerate(deferred):
                            key = min(j * stride + 1, nstep - 1)
                            p3_at.setdefault(key, []).append(em)
                    step_no = 0

                    for pair in range(NPAIR):
                        cc = pair
                        pv = ps_pv.tile([128, QB], F32)
                        dn = ps_dn.tile([128, QB], F32)
                        ss_t = [None] * n
                        ex_t = [None] * n

                        def emit_s(idx):
                            kc = kcs[idx]
                            qlo = _QLO[(qc, kc)]
                            ss = ps_s.tile([128, 2, QB], F32, tag="ss",
                                           name="ss")
                            ss_t[idx] = ss
                            qsl = slice(qc * QB + qlo, (qc + 1) * QB)
                            ks = slice(kc * KB, (kc + 1) * KB)
                            nc.tensor.matmul(
                                ss[:, 0, qlo:], KT[0:64, cc, ks],
                                QT[0:64, cc, qsl], start=True, stop=True,
                            )
                            nc.tensor.matmul(
                                ss[:, 1, qlo:], KT[64:128, cc, ks],
                                QT[64:128, cc, qsl], start=True, stop=True,
                            )

                        def emit_e(idx):
                            kc = kcs[idx]
                            st = _STATUS[(qc, kc)]
                            qlo = _QLO[(qc, kc)]
                            ex = ep.tile([128, 2, QB], BF16, tag="e", name="e")
                            ex_t[idx] = ex
                            nc.scalar.activation(
                                ex[:, :, qlo:], ss_t[idx][:, :, qlo:], ExpF
                            )
                            if st != "full":
                                nc.vector.tensor_mul(
                                    ex[:], ex[:], mk_sb[:, st, :, :]
                                )

                        def emit_p(idx):
                            kc = kcs[idx]
                            ex = ex_t[idx]
                            first, last = idx == 0, idx == n - 1
                            for h2 in range(2):
                                h = 2 * pair + h2
                                nc.tensor.matmul(
                                    pv[h2 * 64:(h2 + 1) * 64, :],
                                    Vx[:, kc, h, :], ex[:, h2, :],
                                    start=first, stop=last,
                                    skip_group_check=True,
                                )
                                nc.tensor.matmul(
                                    dn[h2 * 64:(h2 + 1) * 64, :],
                                    ones64[:], ex[:, h2, :],
                                    start=first, stop=last,
                                    skip_group_check=True,
                                )

                        emit_s(0)
                        emit_e(0)
                        for idx in range(n):
                            if idx + 1 < n:
                                emit_s(idx + 1)
                                emit_e(idx + 1)
                            emit_p(idx)
                            if qc == 0 and step_no in v_at:
                                emit_v(v_at.pop(step_no))
                            for em in p3_at.pop(step_no, ()):
                                em()
                            step_no += 1

                        rc = rp.tile([128, QB], F32, tag="rc", name="rc")
                        nc.vector.reciprocal_approx_fast(rc[:], dn[:])
                        nc.vector.tensor_mul(AT[:, cc, qs], pv[:], rc[:])

                    for tk in sorted(v_at.values()) if qc == 0 else ():
                        emit_v(tk)  # stragglers (shouldn't happen)
                    if qc == 0:
                        v_at = {}
                    for ems in p3_at.values():  # any stragglers
                        for em in ems:
                            em()
                    deferred = make_p3(qc)

                for em in deferred:  # last query block's out-proj tail
                    em()

    nc.finalize()
    return nc


_NC = None


def _get_nc():
    global _NC
    if _NC is None:
        _NC = _build()
    return _NC


def _make_in_maps(hidden_states, Wq, bq, Wk, Wv, Wo):
    hs = np.asarray(hidden_states, np.float32)
    Wq = np.asarray(Wq, np.float32)
    Wk = np.asarray(Wk, np.float32)
    Wv = np.asarray(Wv, np.float32)
    Wo = np.asarray(Wo, np.float32)
    bq = np.asarray(bq, np.float32)

    wq_s = (Wq * np.float32(SCALE)).astype(BF16NP)
    wk_b = Wk.astype(BF16NP)
    wv_b = Wv.astype(BF16NP)
    wo_b = Wo.astype(BF16NP)

    in_maps = []
    for core in range(NCORES):
        b, hh = core // 2, core % 2
        sl = slice(hh * CH, (hh + 1) * CH)
        hsT_pad = np.zeros((D, TP), BF16NP)
        hsT_pad[:, :T] = hs[b].T.astype(BF16NP)
        in_maps.append({
            "hsT": hsT_pad,
            "wq": np.ascontiguousarray(wq_s[:, sl]),
            "wk": np.ascontiguousarray(wk_b[:, sl]),
            "wv": np.ascontiguousarray(wv_b[:, sl]),
            "bqs": np.ascontiguousarray(bq[sl] * np.float32(SCALE)),
            "wo": np.ascontiguousarray(wo_b[sl, :]),
            "maskT": _EXPMASK,
        })
    return in_maps


def _assemble(results, bv, Wo, bo):
    c0 = (np.asarray(bv, np.float32) @ np.asarray(Wo, np.float32)
          + np.asarray(bo, np.float32))
    out = np.empty((B, T, D), np.float32)
    for b in range(B):
        out[b] = (results[2 * b]["outT"][:, :T].T
                  + results[2 * b + 1]["outT"][:, :T].T + c0)
    return out


def kernel(hidden_states, Wq, bq, Wk, Wv, bv, Wo, bo):
    in_maps = _make_in_maps(hidden_states, Wq, bq, Wk, Wv, Wo)
    res = bass_utils.run_bass_kernel_spmd(
        _get_nc(), in_maps, core_ids=list(range(NCORES))
    )
    return _assemble(res.results, bv, Wo, bo)


# revision 19
# speedup vs baseline: 1.1149x; 1.0560x over previous
"""Chunk-causal Whisper attention (B=4, T=1500, D=1024, H=16) on 8 NeuronCores.

Sharding: core c = (batch b = c//2, head-half hh = c%2). Each core runs one
batch element with 8 of the 16 heads (512 of 1024 channels). All matmul
operands are bf16 (PSUM accumulation stays fp32); rel tolerance 2e-2 leaves
ample room. All on-chip tensors are kept transposed: scoresT[k,q] = K @ Q^T
per head, so the softmax reduction runs along the partition (key) axis.

Phase 2 processes heads in pairs (even head on partitions 0-63, odd head on
64-127 of each 128-channel chunk, matching the Q/K projection layout):
  - scores: two row-tiled matmuls (64-row contraction each) run concurrently
    on the two halves of the PE array into one 2-bank PSUM tile.
  - exp: one ACT instruction over the pair, trimmed to the query range that
    has any allowed key in the block (host-precomputed qlo per block).
  - mask: multiplicative {1,0} bf16 mask applied AFTER exp (fast 2-byte DVE
    mode); it also zeroes the stale sub-qlo range of the ex tile.
  - PV + denominator: per head, two col-tiled matmuls run concurrently:
    V (cols 0-63) -> pv tile, ones (cols 64-127) -> den tile broadcast over
    64 partitions. Even head lands at partitions 0-63, odd at 64-127, so one
    reciprocal + one multiply normalizes the whole pair at matching lanes.
Scores are emitted one kc-block ahead of PV so the PE never starves (keeps
the HAM clock gate warm at 2.4 GHz; the fp32r baseline sat throttled at
1.2 GHz for the entire attention phase).

T is padded 1500 -> 1536 with zeros (hsT columns): padded keys contribute
nothing (V rows are zero and partial-block masks zero their exp values);
padded query columns are finite garbage and dropped on the host. bv folds
through softmax (probabilities sum to 1), so the host adds (bv @ Wo + bo).
"""

import sys

import numpy as np
import ml_dtypes

if "/opt/trn_rl_repo" not in sys.path:
    sys.path.insert(0, "/opt/trn_rl_repo")

import concourse.tile as tile  # noqa: E402
from concourse import bacc, mybir  # noqa: E402
import concourse.bass_utils as bass_utils  # noqa: E402

B, T, D = 4, 1500, 1024
H, HD = 16, 64
CHUNK, LOOK = 100, 50
TP = 1536          # padded sequence length
CH = 512           # channels per core (8 heads)
HL = 8             # heads per core
NPAIR = HL // 2
NCORES = 8
SCALE = HD ** -0.5
QB = 512           # query block (matmul moving free dim)
KB = 128           # key block (contraction tile)
NQC = TP // QB     # 3
NKC = TP // KB     # 12
NDC = D // 128     # 8
NCC = CH // 128    # 4
NOC = D // 128     # 8 output-column chunks
F32 = mybir.dt.float32
BF16 = mybir.dt.bfloat16
BF16NP = ml_dtypes.bfloat16


def _jmax(i):
    """Largest key index query i may attend to."""
    return max((i // CHUNK) * CHUNK + CHUNK - 1, i + LOOK)


def _classify():
    """Per (qc, kc) block: status 'skip' | 'full' | mask-index, plus the
    first query column with any allowed key (qlo, floored to 8)."""
    status = {}
    qlos = {}
    masks = []
    for qc in range(NQC):
        q0 = qc * QB
        hi = max(_jmax(min(q0 + ii, T - 1)) for ii in range(QB))
        lo = _jmax(q0)
        for kc in range(NKC):
            k0 = kc * KB
            if k0 > hi:
                status[(qc, kc)] = "skip"
                continue
            if k0 + KB - 1 <= lo and k0 + KB <= T:
                status[(qc, kc)] = "full"
                qlos[(qc, kc)] = 0
                continue
            m = np.zeros((KB, QB), np.float32)
            qlo = QB
            for ii in range(QB):
                i = min(q0 + ii, T - 1)  # padded queries reuse the last row
                n_ok = min(min(_jmax(i), T - 1) + 1 - k0, KB)
                if n_ok > 0:
                    m[:n_ok, ii] = 1.0
                    qlo = min(qlo, ii)
            status[(qc, kc)] = len(masks)
            qlos[(qc, kc)] = (qlo // 8) * 8
            masks.append(m)
    return status, qlos, masks


_STATUS, _QLO, _MASKS = _classify()
NPART = len(_MASKS)

# [128, NPART, 2, QB] multiplicative bf16 mask, duplicated for the head pair.
_EXPMASK = np.ascontiguousarray(
    np.broadcast_to(np.stack(_MASKS)[:, :, None, :], (NPART, KB, 2, QB))
    .transpose(1, 0, 2, 3)
).astype(BF16NP)


def _build():
    nc = bacc.Bacc("TRN2", target_bir_lowering=False, debug=False)
    hsT = nc.dram_tensor("hsT", [D, TP], BF16, kind="ExternalInput")[:]
    wq = nc.dram_tensor("wq", [D, CH], BF16, kind="ExternalInput")[:]
    wk = nc.dram_tensor("wk", [D, CH], BF16, kind="ExternalInput")[:]
    wv = nc.dram_tensor("wv", [D, CH], BF16, kind="ExternalInput")[:]
    bqs = nc.dram_tensor("bqs", [CH], F32, kind="ExternalInput")[:]
    wo = nc.dram_tensor("wo", [CH, D], BF16, kind="ExternalInput")[:]
    maskT = nc.dram_tensor("maskT", [KB, NPART, 2, QB], BF16,
                           kind="ExternalInput")[:]
    outT = nc.dram_tensor("outT", [D, TP], BF16, kind="ExternalOutput")[:]

    hsT_r = hsT.rearrange("(a p) t -> a p t", p=128)
    wq_r = wq.rearrange("(a p) c -> a p c", p=128)
    wk_r = wk.rearrange("(a p) c -> a p c", p=128)
    wv_r = wv.rearrange("(a p) c -> a p c", p=128)
    wo_r = wo.rearrange("(a p) o -> a p o", p=128)
    outT_r = outT.rearrange("(a p) t -> a p t", p=128)

    ExpF = mybir.ActivationFunctionType.Exp

    with tile.TileContext(nc) as tc, tc.tile_pool(name="per", bufs=1) as per:
        KT = per.tile([128, NCC, TP], BF16)        # K^T: [c, cc, t]
        QT = per.tile([128, NCC, TP], BF16)        # Q^T (scale+bias folded)
        Vx = per.tile([128, NKC, HL, HD], BF16)    # V: [k, kc, h, hd]
        AT = per.tile([128, NCC, TP], BF16)        # normalized attnT
        ones64 = per.tile([128, HD], BF16)
        bq_sb = per.tile([128, NCC], F32)
        mk_sb = per.tile([128, NPART, 2, QB], BF16)
        wo_sb = per.tile([128, NCC, D], BF16)
        nc.vector.memset(ones64[:], 1.0)
        nc.sync.dma_start(bq_sb[:], bqs.rearrange("(a p) -> p a", p=128))

        # ---------------- phase 1: K/Q projections ----------------
        # V's projection is deferred into phase 2's instruction stream (the
        # PE has slack there while the scalar engine runs exps), so qc0's
        # exps start ~20us earlier. DMA order matters: hs/wk gate the first
        # matmul; the 3.6MB mask only has to land before qc0's first exp.
        with tc.tile_pool(name="p1", bufs=1) as p1, \
             tc.tile_pool(name="w", bufs=3) as wp:
            hs_sb = p1.tile([128, NDC, TP], BF16)
            wk_sb = wp.tile([128, NDC, CH], BF16, tag="w")
            wq_sb = wp.tile([128, NDC, CH], BF16, tag="w")
            wv_sb = wp.tile([128, NDC, CH], BF16, tag="w")
            for dc in range(NDC):
                nc.sync.dma_start(hs_sb[:, dc, :], hsT_r[dc])
                nc.sync.dma_start(wk_sb[:, dc, :], wk_r[dc])
            for dc in range(NDC):
                nc.sync.dma_start(wq_sb[:, dc, :], wq_r[dc])
            for dc in range(NDC):
                nc.sync.dma_start(wv_sb[:, dc, :], wv_r[dc])
            nc.sync.dma_start(mk_sb[:], maskT[:])
            for cc in range(NCC):
                nc.sync.dma_start(wo_sb[:, cc, :], wo_r[cc])

            with tc.tile_pool(name="ps1", bufs=2, space="PSUM") as ps1:
                # K^T[c, t] = sum_d Wk[d, c] * hsT[d, t]
                for tb in range(NQC):
                    ts = slice(tb * QB, (tb + 1) * QB)
                    pss = [ps1.tile([128, QB], F32, tag=f"pp{cc}",
                                    name=f"pp{cc}") for cc in range(NCC)]
                    for dc in range(NDC):
                        for cc in range(NCC):
                            nc.tensor.matmul(
                                pss[cc][:],
                                wk_sb[:, dc, cc * 128:(cc + 1) * 128],
                                hs_sb[:, dc, ts],
                                start=(dc == 0), stop=(dc == NDC - 1),
                            )
                    for cc in range(NCC):
                        nc.scalar.copy(KT[:, cc, ts], pss[cc][:])

                # Q^T[c, t] (wq pre-scaled by 1/sqrt(hd) on host; bias here)
                for tb in range(NQC):
                    ts = slice(tb * QB, (tb + 1) * QB)
                    pss = [ps1.tile([128, QB], F32, tag=f"pp{cc}",
                                    name=f"pp{cc}") for cc in range(NCC)]
                    for dc in range(NDC):
                        for cc in range(NCC):
                            nc.tensor.matmul(
                                pss[cc][:],
                                wq_sb[:, dc, cc * 128:(cc + 1) * 128],
                                hs_sb[:, dc, ts],
                                start=(dc == 0), stop=(dc == NDC - 1),
                            )
                    for cc in range(NCC):
                        nc.vector.tensor_scalar_add(
                            QT[:, cc, ts], pss[cc][:], bq_sb[:, cc:cc + 1]
                        )

            # --------- phases 2+3: attention, out-proj, V-proj ---------
            # qc-outer so AT[:, :, qs] completes per query block; each query
            # block's 32 output-projection matmuls are then drip-fed into the
            # NEXT query block's scalar-bound step stream (the PE has slack
            # there), instead of running as a serial PE-only tail. The V
            # projection is likewise dripped into qc0's steps (V psum shares
            # the score pool; copies go to vector/gpsimd, keeping scalar for
            # exps).
            with tc.tile_pool(name="ex", bufs=3) as ep, \
                 tc.tile_pool(name="rcp", bufs=2) as rp, \
                 tc.tile_pool(name="fin", bufs=2) as fp, \
                 tc.tile_pool(name="ps_s", bufs=2, space="PSUM") as ps_s, \
                 tc.tile_pool(name="ps_pv", bufs=2, space="PSUM") as ps_pv, \
                 tc.tile_pool(name="ps_dn", bufs=2, space="PSUM") as ps_dn:

                def emit_v(tk):
                    pot = ps_s.tile([128, 2, QB], F32, tag="ss", name="pv1")
                    psv = pot[:, 0, :]
                    for dc in range(NDC):
                        nc.tensor.matmul(
                            psv,
                            hs_sb[:, dc, tk * KB:(tk + 1) * KB],
                            wv_sb[:, dc, :],
                            start=(dc == 0), stop=(dc == NDC - 1),
                            skip_group_check=True,
                        )
                    nc.vector.tensor_copy(
                        Vx[:, tk, :, :],
                        psv.rearrange("p (h d) -> p h d", d=HD),
                    )

                def make_p3(qc):
                    """Per-oc emitters for query block qc's out-proj."""
                    qs = slice(qc * QB, (qc + 1) * QB)
                    fin = fp.tile([128, NOC, QB], BF16, tag="fin", name="fin")

                    def emit_oc(oc):
                        pot = ps_s.tile([128, 2, QB], F32, tag="ss", name="po")
                        po = pot[:, 0, :]
                        for cc in range(NCC):
                            nc.tensor.matmul(
                                po,
                                wo_sb[:, cc, oc * 128:(oc + 1) * 128],
                                AT[:, cc, qs],
                                start=(cc == 0), stop=(cc == NCC - 1),
                            )
                        nc.vector.tensor_copy(fin[:, oc, :], po)
                        nc.sync.dma_start(outT_r[oc][:, qs], fin[:, oc, :])

                    return [lambda oc=oc: emit_oc(oc) for oc in range(NOC)]

                # Flat, globally software-pipelined step stream: scores+exp
                # run a fixed lookahead ahead of PV across pair/qc boundaries
                # so neither the PE nor the scalar engine ever drains at a
                # segment edge.
                steps = []
                qc_kcs = {}
                for qc in range(NQC):
                    kcs = [kc for kc in range(NKC)
                           if _STATUS[(qc, kc)] != "skip"]
                    qc_kcs[qc] = kcs
                    for pair in range(NPAIR):
                        for idx in range(len(kcs)):
                            steps.append((qc, pair, idx))
                nsteps = len(steps)
                qc_start = {qc: min(i for i, s in enumerate(steps)
                                    if s[0] == qc) for qc in range(NQC)}
                qc_end = {qc: max(i for i, s in enumerate(steps)
                                  if s[0] == qc) for qc in range(NQC)}

                # V-projection blocks tk 5..11 dripped into early P-steps
                # (tk 0..4 are primed before the first PV); out-proj of each
                # qc dripped over the next qc's P-steps.
                v_at = {2 * j + 2: tk for j, tk in enumerate(range(5, NKC))}
                p3_at = {}
                for qc in range(NQC - 1):
                    lo, hi = qc_start[qc + 1], qc_end[qc + 1]
                    stride = max(2, (hi - lo) // NOC)
                    for oc in range(NOC):
                        key = min(lo + oc * stride + 1, hi)
                        p3_at.setdefault(key, []).append((qc, oc))
                p3_emitters = {}

                ss_t = {}
                ex_t = {}
                pvdn = {}

                def emit_se(i):
                    qc, pair, idx = steps[i]
                    kc = qc_kcs[qc][idx]
                    cc = pair
                    st = _STATUS[(qc, kc)]
                    qlo = _QLO[(qc, kc)]
                    ss = ps_s.tile([128, 2, QB], F32, tag="ss", name="ss")
                    qsl = slice(qc * QB + qlo, (qc + 1) * QB)
                    ks = slice(kc * KB, (kc + 1) * KB)
                    nc.tensor.matmul(
                        ss[:, 0, qlo:], KT[0:64, cc, ks],
                        QT[0:64, cc, qsl], start=True, stop=True,
                    )
                    nc.tensor.matmul(
                        ss[:, 1, qlo:], KT[64:128, cc, ks],
                        QT[64:128, cc, qsl], start=True, stop=True,
                    )
                    ex = ep.tile([128, 2, QB], BF16, tag="e", name="e")
                    nc.scalar.activation(ex[:, :, qlo:], ss[:, :, qlo:], ExpF)
                    if st != "full":
                        nc.vector.tensor_mul(
                            ex[:, :, qlo:], ex[:, :, qlo:],
                            mk_sb[:, st, :, qlo:],
                        )
                    ex_t[i] = ex

                def emit_p(i):
                    qc, pair, idx = steps[i]
                    kcs = qc_kcs[qc]
                    kc = kcs[idx]
                    n = len(kcs)
                    if idx == 0:
                        pv = ps_pv.tile([128, QB], F32, name="pv")
                        dn = ps_dn.tile([128, QB], F32, name="dn")
                        pvdn[(qc, pair)] = (pv, dn)
                    pv, dn = pvdn[(qc, pair)]
                    ex = ex_t.pop(i)
                    first, last = idx == 0, idx == n - 1
                    # Only queries >= qlo have allowed keys in this block;
                    # the first block of each (qc, pair) has qlo == 0, so the
                    # start=True matmul initializes the whole pv/dn range and
                    # later blocks accumulate into their [qlo:] slice only.
                    qlo = _QLO[(qc, kc)]
                    assert not first or qlo == 0
                    for h2 in range(2):
                        h = 2 * pair + h2
                        nc.tensor.matmul(
                            pv[h2 * 64:(h2 + 1) * 64, qlo:],
                            Vx[:, kc, h, :], ex[:, h2, qlo:],
                            start=first, stop=last,
                            skip_group_check=True,
                        )
                        nc.tensor.matmul(
                            dn[h2 * 64:(h2 + 1) * 64, qlo:],
                            ones64[:], ex[:, h2, qlo:],
                            start=first, stop=last,
                            skip_group_check=True,
                        )
                    if last:
                        qs = slice(qc * QB, (qc + 1) * QB)
                        rc = rp.tile([128, QB], F32, tag="rc", name="rc")
                        nc.vector.reciprocal_approx_fast(rc[:], dn[:])
                        nc.vector.tensor_mul(AT[:, pair, qs], pv[:], rc[:])

                LOOK_AHEAD = 2
                emit_se(0)
                emit_se(1)
                for tk in range(5):  # qc0's PV blocks need kc <= 4
                    emit_v(tk)
                se = 2
                for p in range(nsteps):
                    emit_p(p)
                    if p in v_at:
                        emit_v(v_at.pop(p))
                    for qc, oc in p3_at.pop(p, ()):
                        if qc not in p3_emitters:
                            p3_emitters[qc] = make_p3(qc)
                        p3_emitters[qc][oc]()
                    while se < min(p + 1 + LOOK_AHEAD, nsteps):
                        emit_se(se)
                        se += 1

                for tk in sorted(v_at.values()):
                    emit_v(tk)  # stragglers (shouldn't happen)
                for em in make_p3(NQC - 1):  # last query block's out-proj
                    em()

    nc.finalize()
    return nc


_NC = None


def _get_nc():
    global _NC
    if _NC is None:
        _NC = _build()
    return _NC


def _make_in_maps(hidden_states, Wq, bq, Wk, Wv, Wo):
    hs = np.asarray(hidden_states, np.float32)
    Wq = np.asarray(Wq, np.float32)
    Wk = np.asarray(Wk, np.float32)
    Wv = np.asarray(Wv, np.float32)
    Wo = np.asarray(Wo, np.float32)
    bq = np.asarray(bq, np.float32)

    wq_s = (Wq * np.float32(SCALE)).astype(BF16NP)
    wk_b = Wk.astype(BF16NP)
    wv_b = Wv.astype(BF16NP)
    wo_b = Wo.astype(BF16NP)

    in_maps = []
    for core in range(NCORES):
        b, hh = core // 2, core % 2
        sl = slice(hh * CH, (hh + 1) * CH)
        hsT_pad = np.zeros((D, TP), BF16NP)
        hsT_pad[:, :T] = hs[b].T.astype(BF16NP)
        in_maps.append({
            "hsT": hsT_pad,
            "wq": np.ascontiguousarray(wq_s[:, sl]),
            "wk": np.ascontiguousarray(wk_b[:, sl]),
            "wv": np.ascontiguousarray(wv_b[:, sl]),
            "bqs": np.ascontiguousarray(bq[sl] * np.float32(SCALE)),
            "wo": np.ascontiguousarray(wo_b[sl, :]),
            "maskT": _EXPMASK,
        })
    return in_maps


def _assemble(results, bv, Wo, bo):
    c0 = (np.asarray(bv, np.float32) @ np.asarray(Wo, np.float32)
          + np.asarray(bo, np.float32))
    out = np.empty((B, T, D), np.float32)
    for b in range(B):
        out[b] = (results[2 * b]["outT"][:, :T].T
                  + results[2 * b + 1]["outT"][:, :T].T + c0)
    return out


def kernel(hidden_states, Wq, bq, Wk, Wv, bv, Wo, bo):
    in_maps = _make_in_maps(hidden_states, Wq, bq, Wk, Wv, Wo)
    res = bass_utils.run_bass_kernel_spmd(
        _get_nc(), in_maps, core_ids=list(range(NCORES))
    )
    return _assemble(res.results, bv, Wo, bo)


# revision 24
# speedup vs baseline: 1.1246x; 1.0087x over previous
"""Chunk-causal Whisper attention (B=4, T=1500, D=1024, H=16) on 8 NeuronCores.

Sharding: core c = (batch b = c//2, head-half hh = c%2). Each core runs one
batch element with 8 of the 16 heads (512 of 1024 channels). All matmul
operands are bf16 (PSUM accumulation stays fp32); rel tolerance 2e-2 leaves
ample room. All on-chip tensors are kept transposed: scoresT[k,q] = K @ Q^T
per head, so the softmax reduction runs along the partition (key) axis.

Phase 2 processes heads in pairs (even head on partitions 0-63, odd head on
64-127 of each 128-channel chunk, matching the Q/K projection layout):
  - scores: two row-tiled matmuls (64-row contraction each) run concurrently
    on the two halves of the PE array into one 2-bank PSUM tile.
  - exp: one ACT instruction over the pair, trimmed to the query range that
    has any allowed key in the block (host-precomputed qlo per block).
  - mask: multiplicative {1,0} bf16 mask applied AFTER exp (fast 2-byte DVE
    mode); it also zeroes the stale sub-qlo range of the ex tile.
  - PV + denominator: per head, two col-tiled matmuls run concurrently:
    V (cols 0-63) -> pv tile, ones (cols 64-127) -> den tile broadcast over
    64 partitions. Even head lands at partitions 0-63, odd at 64-127, so one
    reciprocal + one multiply normalizes the whole pair at matching lanes.
Scores are emitted one kc-block ahead of PV so the PE never starves (keeps
the HAM clock gate warm at 2.4 GHz; the fp32r baseline sat throttled at
1.2 GHz for the entire attention phase).

T is padded 1500 -> 1536 with zeros (hsT columns): padded keys contribute
nothing (V rows are zero and partial-block masks zero their exp values);
padded query columns are finite garbage and dropped on the host. bv folds
through softmax (probabilities sum to 1), so the host adds (bv @ Wo + bo).
"""

import sys

import numpy as np
import ml_dtypes

if "/opt/trn_rl_repo" not in sys.path:
    sys.path.insert(0, "/opt/trn_rl_repo")

import concourse.tile as tile  # noqa: E402
from concourse import bacc, mybir  # noqa: E402
import concourse.bass_utils as bass_utils  # noqa: E402

B, T, D = 4, 1500, 1024
H, HD = 16, 64
CHUNK, LOOK = 100, 50
TP = 1536          # padded sequence length
CH = 512           # channels per core (8 heads)
HL = 8             # heads per core
NPAIR = HL // 2
NCORES = 8
SCALE = HD ** -0.5
QB = 512           # query block (matmul moving free dim)
KB = 128           # key block (contraction tile)
NQC = TP // QB     # 3
NKC = TP // KB     # 12
NDC = D // 128     # 8
NCC = CH // 128    # 4
NOC = D // 128     # 8 output-column chunks
F32 = mybir.dt.float32
BF16 = mybir.dt.bfloat16
BF16NP = ml_dtypes.bfloat16


def _jmax(i):
    """Largest key index query i may attend to."""
    return max((i // CHUNK) * CHUNK + CHUNK - 1, i + LOOK)


def _classify():
    """Per (qc, kc) block: status 'skip' | 'full' | mask-index, plus the
    first query column with any allowed key (qlo, floored to 8)."""
    status = {}
    qlos = {}
    masks = []
    for qc in range(NQC):
        q0 = qc * QB
        hi = max(_jmax(min(q0 + ii, T - 1)) for ii in range(QB))
        lo = _jmax(q0)
        for kc in range(NKC):
            k0 = kc * KB
            if k0 > hi:
                status[(qc, kc)] = "skip"
                continue
            if k0 + KB - 1 <= lo and k0 + KB <= T:
                status[(qc, kc)] = "full"
                qlos[(qc, kc)] = 0
                continue
            m = np.zeros((KB, QB), np.float32)
            qlo = QB
            for ii in range(QB):
                i = min(q0 + ii, T - 1)  # padded queries reuse the last row
                n_ok = min(min(_jmax(i), T - 1) + 1 - k0, KB)
                if n_ok > 0:
                    m[:n_ok, ii] = 1.0
                    qlo = min(qlo, ii)
            status[(qc, kc)] = len(masks)
            qlos[(qc, kc)] = (qlo // 8) * 8
            masks.append(m)
    return status, qlos, masks


_STATUS, _QLO, _MASKS = _classify()
NPART = len(_MASKS)

# Last real query column per block (qc2 holds only 476 real queries; the
# padded remainder is never computed, never read, and dropped on the host).
_QHI = {qc: min(QB, -(-(T - qc * QB) // 8) * 8) for qc in range(NQC)}

# [128, NPART, 2, QB] multiplicative bf16 mask, duplicated for the head pair.
_EXPMASK = np.ascontiguousarray(
    np.broadcast_to(np.stack(_MASKS)[:, :, None, :], (NPART, KB, 2, QB))
    .transpose(1, 0, 2, 3)
).astype(BF16NP)


def _build():
    nc = bacc.Bacc("TRN2", target_bir_lowering=False, debug=False)
    hsT = nc.dram_tensor("hsT", [D, TP], BF16, kind="ExternalInput")[:]
    wq = nc.dram_tensor("wq", [D, CH], BF16, kind="ExternalInput")[:]
    wk = nc.dram_tensor("wk", [D, CH], BF16, kind="ExternalInput")[:]
    wv = nc.dram_tensor("wv", [D, CH], BF16, kind="ExternalInput")[:]
    bqs = nc.dram_tensor("bqs", [CH], F32, kind="ExternalInput")[:]
    wo = nc.dram_tensor("wo", [CH, D], BF16, kind="ExternalInput")[:]
    maskT = nc.dram_tensor("maskT", [KB, NPART, 2, QB], BF16,
                           kind="ExternalInput")[:]
    outT = nc.dram_tensor("outT", [D, TP], BF16, kind="ExternalOutput")[:]

    hsT_r = hsT.rearrange("(a p) t -> a p t", p=128)
    wq_r = wq.rearrange("(a p) c -> a p c", p=128)
    wk_r = wk.rearrange("(a p) c -> a p c", p=128)
    wv_r = wv.rearrange("(a p) c -> a p c", p=128)
    wo_r = wo.rearrange("(a p) o -> a p o", p=128)
    outT_r = outT.rearrange("(a p) t -> a p t", p=128)

    ExpF = mybir.ActivationFunctionType.Exp

    with tile.TileContext(nc) as tc, tc.tile_pool(name="per", bufs=1) as per:
        KT = per.tile([128, NCC, TP], BF16)        # K^T: [c, cc, t]
        QT = per.tile([128, NCC, TP], BF16)        # Q^T (scale+bias folded)
        Vx = per.tile([128, NKC, HL, HD], BF16)    # V: [k, kc, h, hd]
        AT = per.tile([128, NCC, TP], BF16)        # normalized attnT
        ones64 = per.tile([128, HD], BF16)
        bq_sb = per.tile([128, NCC], F32)
        mk_sb = per.tile([128, NPART, 2, QB], BF16)
        wo_sb = per.tile([128, NCC, D], BF16)
        nc.vector.memset(ones64[:], 1.0)
        nc.sync.dma_start(bq_sb[:], bqs.rearrange("(a p) -> p a", p=128))

        # ---------------- phase 1: K/Q projections ----------------
        # V's projection is deferred into phase 2's instruction stream (the
        # PE has slack there while the scalar engine runs exps), so qc0's
        # exps start ~20us earlier. DMA order matters: hs/wk gate the first
        # matmul; the 3.6MB mask only has to land before qc0's first exp.
        with tc.tile_pool(name="p1", bufs=1) as p1, \
             tc.tile_pool(name="w", bufs=3) as wp:
            hs_sb = p1.tile([128, NDC, TP], BF16)
            wk_sb = wp.tile([128, NDC, CH], BF16, tag="w")
            wq_sb = wp.tile([128, NDC, CH], BF16, tag="w")
            wv_sb = wp.tile([128, NDC, CH], BF16, tag="w")
            for dc in range(NDC):
                nc.sync.dma_start(hs_sb[:, dc, :], hsT_r[dc])
                nc.sync.dma_start(wk_sb[:, dc, :], wk_r[dc])
            for dc in range(NDC):
                nc.sync.dma_start(wq_sb[:, dc, :], wq_r[dc])
            for dc in range(NDC):
                nc.sync.dma_start(wv_sb[:, dc, :], wv_r[dc])
            nc.sync.dma_start(mk_sb[:], maskT[:])
            for cc in range(NCC):
                nc.sync.dma_start(wo_sb[:, cc, :], wo_r[cc])

            with tc.tile_pool(name="ps1", bufs=2, space="PSUM") as ps1:
                # K^T[c, t] = sum_d Wk[d, c] * hsT[d, t]
                for tb in range(NQC):
                    ts = slice(tb * QB, (tb + 1) * QB)
                    pss = [ps1.tile([128, QB], F32, tag=f"pp{cc}",
                                    name=f"pp{cc}") for cc in range(NCC)]
                    for dc in range(NDC):
                        for cc in range(NCC):
                            nc.tensor.matmul(
                                pss[cc][:],
                                wk_sb[:, dc, cc * 128:(cc + 1) * 128],
                                hs_sb[:, dc, ts],
                                start=(dc == 0), stop=(dc == NDC - 1),
                            )
                    for cc in range(NCC):
                        nc.scalar.copy(KT[:, cc, ts], pss[cc][:])

                # Q^T[c, t] (wq pre-scaled by 1/sqrt(hd) on host; bias here)
                for tb in range(NQC):
                    ts = slice(tb * QB, (tb + 1) * QB)
                    pss = [ps1.tile([128, QB], F32, tag=f"pp{cc}",
                                    name=f"pp{cc}") for cc in range(NCC)]
                    for dc in range(NDC):
                        for cc in range(NCC):
                            nc.tensor.matmul(
                                pss[cc][:],
                                wq_sb[:, dc, cc * 128:(cc + 1) * 128],
                                hs_sb[:, dc, ts],
                                start=(dc == 0), stop=(dc == NDC - 1),
                            )
                    for cc in range(NCC):
                        nc.vector.tensor_scalar_add(
                            QT[:, cc, ts], pss[cc][:], bq_sb[:, cc:cc + 1]
                        )

            # --------- phases 2+3: attention, out-proj, V-proj ---------
            # qc-outer so AT[:, :, qs] completes per query block; each query
            # block's 32 output-projection matmuls are then drip-fed into the
            # NEXT query block's scalar-bound step stream (the PE has slack
            # there), instead of running as a serial PE-only tail. The V
            # projection is likewise dripped into qc0's steps (V psum shares
            # the score pool; copies go to vector/gpsimd, keeping scalar for
            # exps).
            with tc.tile_pool(name="ex", bufs=3) as ep, \
                 tc.tile_pool(name="rcp", bufs=2) as rp, \
                 tc.tile_pool(name="fin", bufs=2) as fp, \
                 tc.tile_pool(name="ps_s", bufs=2, space="PSUM") as ps_s, \
                 tc.tile_pool(name="ps_pv", bufs=2, space="PSUM") as ps_pv, \
                 tc.tile_pool(name="ps_dn", bufs=2, space="PSUM") as ps_dn:

                def emit_v(tk):
                    pot = ps_s.tile([128, 2, QB], F32, tag="ss", name="pv1")
                    psv = pot[:, 0, :]
                    for dc in range(NDC):
                        nc.tensor.matmul(
                            psv,
                            hs_sb[:, dc, tk * KB:(tk + 1) * KB],
                            wv_sb[:, dc, :],
                            start=(dc == 0), stop=(dc == NDC - 1),
                            skip_group_check=True,
                        )
                    nc.vector.tensor_copy(
                        Vx[:, tk, :, :],
                        psv.rearrange("p (h d) -> p h d", d=HD),
                    )

                def make_p3(qc):
                    """Per-oc emitters for query block qc's out-proj."""
                    qhi = _QHI[qc]
                    qs = slice(qc * QB, qc * QB + qhi)
                    fin = fp.tile([128, NOC, QB], BF16, tag="fin", name="fin")

                    def emit_oc(oc):
                        pot = ps_s.tile([128, 2, QB], F32, tag="ss", name="po")
                        po = pot[:, 0, :qhi]
                        for cc in range(NCC):
                            nc.tensor.matmul(
                                po,
                                wo_sb[:, cc, oc * 128:(oc + 1) * 128],
                                AT[:, cc, qs],
                                start=(cc == 0), stop=(cc == NCC - 1),
                            )
                        nc.vector.tensor_copy(fin[:, oc, :qhi], po)
                        nc.sync.dma_start(outT_r[oc][:, qs], fin[:, oc, :qhi])

                    return [lambda oc=oc: emit_oc(oc) for oc in range(NOC)]

                # Flat, globally software-pipelined step stream: scores+exp
                # run a fixed lookahead ahead of PV across pair/qc boundaries
                # so neither the PE nor the scalar engine ever drains at a
                # segment edge.
                steps = []
                qc_kcs = {}
                for qc in range(NQC):
                    kcs = [kc for kc in range(NKC)
                           if _STATUS[(qc, kc)] != "skip"]
                    qc_kcs[qc] = kcs
                    for pair in range(NPAIR):
                        for idx in range(len(kcs)):
                            steps.append((qc, pair, idx))
                nsteps = len(steps)
                qc_start = {qc: min(i for i, s in enumerate(steps)
                                    if s[0] == qc) for qc in range(NQC)}
                qc_end = {qc: max(i for i, s in enumerate(steps)
                                  if s[0] == qc) for qc in range(NQC)}

                # V-projection blocks tk 5..11 dripped into early P-steps
                # (tk 0..4 are primed before the first PV); out-proj of each
                # qc dripped over the next qc's P-steps.
                v_at = {2 * j + 2: tk for j, tk in enumerate(range(5, NKC))}
                p3_at = {}
                for qc in range(NQC - 1):
                    lo, hi = qc_start[qc + 1], qc_end[qc + 1]
                    stride = max(2, (hi - lo) // NOC)
                    for oc in range(NOC):
                        key = min(lo + oc * stride + 1, hi)
                        p3_at.setdefault(key, []).append((qc, oc))
                p3_emitters = {}

                ss_t = {}
                ex_t = {}
                pvdn = {}

                def emit_se(i):
                    qc, pair, idx = steps[i]
                    kc = qc_kcs[qc][idx]
                    cc = pair
                    st = _STATUS[(qc, kc)]
                    qlo = _QLO[(qc, kc)]
                    qhi = _QHI[qc]
                    ss = ps_s.tile([128, 2, QB], F32, tag="ss", name="ss")
                    qsl = slice(qc * QB + qlo, qc * QB + qhi)
                    ks = slice(kc * KB, (kc + 1) * KB)
                    nc.tensor.matmul(
                        ss[:, 0, qlo:qhi], KT[0:64, cc, ks],
                        QT[0:64, cc, qsl], start=True, stop=True,
                    )
                    nc.tensor.matmul(
                        ss[:, 1, qlo:qhi], KT[64:128, cc, ks],
                        QT[64:128, cc, qsl], start=True, stop=True,
                    )
                    ex = ep.tile([128, 2, QB], BF16, tag="e", name="e")
                    nc.scalar.activation(
                        ex[:, :, qlo:qhi], ss[:, :, qlo:qhi], ExpF
                    )
                    if st != "full":
                        nc.vector.tensor_mul(
                            ex[:, :, qlo:qhi], ex[:, :, qlo:qhi],
                            mk_sb[:, st, :, qlo:qhi],
                        )
                    ex_t[i] = ex

                def emit_p(i):
                    qc, pair, idx = steps[i]
                    kcs = qc_kcs[qc]
                    kc = kcs[idx]
                    n = len(kcs)
                    if idx == 0:
                        pv = ps_pv.tile([128, QB], F32, name="pv")
                        dn = ps_dn.tile([128, QB], F32, name="dn")
                        pvdn[(qc, pair)] = (pv, dn)
                    pv, dn = pvdn[(qc, pair)]
                    ex = ex_t.pop(i)
                    first, last = idx == 0, idx == n - 1
                    # Only queries >= qlo have allowed keys in this block;
                    # the first block of each (qc, pair) has qlo == 0, so the
                    # start=True matmul initializes the whole pv/dn range and
                    # later blocks accumulate into their [qlo:] slice only.
                    qlo = _QLO[(qc, kc)]
                    qhi = _QHI[qc]
                    assert not first or qlo == 0
                    # pv_even (cols 0-63) and pv_odd (cols 64-127) run
                    # concurrently on the two column halves of the PE array,
                    # then the two den matmuls do the same — emit as two
                    # column-disjoint waves, not interleaved per head.
                    for h2 in range(2):
                        h = 2 * pair + h2
                        nc.tensor.matmul(
                            pv[h2 * 64:(h2 + 1) * 64, qlo:qhi],
                            Vx[:, kc, h, :], ex[:, h2, qlo:qhi],
                            start=first, stop=last,
                            skip_group_check=True,
                        )
                    for h2 in range(2):
                        nc.tensor.matmul(
                            dn[h2 * 64:(h2 + 1) * 64, qlo:qhi],
                            ones64[:], ex[:, h2, qlo:qhi],
                            start=first, stop=last,
                            skip_group_check=True,
                        )
                    if last:
                        qs = slice(qc * QB, qc * QB + qhi)
                        rc = rp.tile([128, QB], F32, tag="rc", name="rc")
                        nc.vector.reciprocal_approx_fast(
                            rc[:, :qhi], dn[:, :qhi]
                        )
                        nc.vector.tensor_mul(
                            AT[:, pair, qs], pv[:, :qhi], rc[:, :qhi]
                        )

                LOOK_AHEAD = 2
                emit_se(0)
                emit_se(1)
                for tk in range(5):  # qc0's PV blocks need kc <= 4
                    emit_v(tk)
                se = 2
                for p in range(nsteps):
                    emit_p(p)
                    if p in v_at:
                        emit_v(v_at.pop(p))
                    for qc, oc in p3_at.pop(p, ()):
                        if qc not in p3_emitters:
                            p3_emitters[qc] = make_p3(qc)
                        p3_emitters[qc][oc]()
                    while se < min(p + 1 + LOOK_AHEAD, nsteps):
                        emit_se(se)
                        se += 1

                for tk in sorted(v_at.values()):
                    emit_v(tk)  # stragglers (shouldn't happen)
                for em in make_p3(NQC - 1):  # last query block's out-proj
                    em()

    nc.finalize()
    return nc


_NC = None


def _get_nc():
    global _NC
    if _NC is None:
        _NC = _build()
    return _NC


def _make_in_maps(hidden_states, Wq, bq, Wk, Wv, Wo):
    hs = np.asarray(hidden_states, np.float32)
    Wq = np.asarray(Wq, np.float32)
    Wk = np.asarray(Wk, np.float32)
    Wv = np.asarray(Wv, np.float32)
    Wo = np.asarray(Wo, np.float32)
    bq = np.asarray(bq, np.float32)

    wq_s = (Wq * np.float32(SCALE)).astype(BF16NP)
    wk_b = Wk.astype(BF16NP)
    wv_b = Wv.astype(BF16NP)
    wo_b = Wo.astype(BF16NP)

    in_maps = []
    for core in range(NCORES):
        b, hh = core // 2, core % 2
        sl = slice(hh * CH, (hh + 1) * CH)
        hsT_pad = np.zeros((D, TP), BF16NP)
        hsT_pad[:, :T] = hs[b].T.astype(BF16NP)
        in_maps.append({
            "hsT": hsT_pad,
            "wq": np.ascontiguousarray(wq_s[:, sl]),
            "wk": np.ascontiguousarray(wk_b[:, sl]),
            "wv": np.ascontiguousarray(wv_b[:, sl]),
            "bqs": np.ascontiguousarray(bq[sl] * np.float32(SCALE)),
            "wo": np.ascontiguousarray(wo_b[sl, :]),
            "maskT": _EXPMASK,
        })
    return in_maps


def _assemble(results, bv, Wo, bo):
    c0 = (np.asarray(bv, np.float32) @ np.asarray(Wo, np.float32)
          + np.asarray(bo, np.float32))
    out = np.empty((B, T, D), np.float32)
    for b in range(B):
        out[b] = (results[2 * b]["outT"][:, :T].T
                  + results[2 * b + 1]["outT"][:, :T].T + c0)
    return out


def kernel(hidden_states, Wq, bq, Wk, Wv, bv, Wo, bo):
    in_maps = _make_in_maps(hidden_states, Wq, bq, Wk, Wv, Wo)
    res = bass_utils.run_bass_kernel_spmd(
        _get_nc(), in_maps, core_ids=list(range(NCORES))
    )
    return _assemble(res.results, bv, Wo, bo)
